# revision 24
# baseline (speedup 1.0000x reference)
"""Trainium2 Bass kernel for the CellLoss problem.

loss = mean_i [ 1/(x[i, l_i] + 0.1) + sum_j x[i,j] * (x[i,j] > x[i, l_i]) ]
with x: [131072, 256] f32, l: [131072] int labels in [0, 256).

Key reformulation: each row's loss is invariant under permuting that
row's 256 class scores, so the host swaps x[i, l_i] <-> x[i, 0] per row
(pure layout prep). The true-class score g then sits at column 0 of
every row: the per-tile gather pass disappears and g ships as a tiny
exact-f32 side tensor (1/(g+0.1) is ill-conditioned near g=-0.1; bf16 g
would cost ~1.4e-2 rel err, measured).

Pure data parallel across 8 NeuronCores (16384 rows each). Per core,
partition p owns rows [p*128, (p+1)*128); tile t is the [128, 256]
block of row p*128+t per partition. Everything reads the f32 x stream
directly - no bf16 cast pass exists (scalar_tensor_tensor runs at the
same 1x rate for f32 and bf16; only its OUTPUT is written bf16).

Margin per tile, engine chosen by PATTERN (cycle of 16):
 "D": DVE stt (x is_gt g) mult x -> masked-x tile (bf16 out).
 "A": ScalarE Relu(x-g) and Sign(x-g) (f32 in, bf16 out, exact f32
      bias); margin_A = sum relu + g*c with c from the sign sums:
      sum_j sign = 2c - 255 (the label ties exactly at 0).
TensorE accumulates the global sums in PSUM: ones^T @ [t|t+1] pairs of
masked-x/relu tiles -> ps_m [1,512]; [g_t|g_t+1]^T @ sign pairs ->
ps_s [2,512] whose junk half-rows are discarded at the tail via
selector-weight matmuls. inv = 1/(g+0.1) is computed up front (overlaps
the DMA fill). Host sums the 8 core partials / B.

DMA: 8 chunks of 2 MiB alternating the two HWDGE rings (sync/scalar).
"""

import numpy as np
from contextlib import ExitStack

import concourse.bass as bass
import concourse.mybir as mybir
import concourse.tile as tile
from concourse.bass_utils import run_bass_kernel_spmd

F32 = mybir.dt.float32
BF16 = mybir.dt.bfloat16

B, C = 131072, 256
N_CORES = 8
B_LOCAL = B // N_CORES          # 16384
P = 128
N_TILES = B_LOCAL // P          # 128
TILES_PER_DMA = 16              # pattern cycle length (A-capable chunks)
# chunk sizes in tiles: small first chunk (starts compute sooner), big
# middles, tiny tail chunks (cheap pipeline drain)
CHUNK_SIZES = [8] + [16] * 7 + [4, 4]

# margin engine per tile, cyclic ("D" DVE stt / "A" ScalarE relu+sign);
# A-tiles must come in adjacent pairs within the cycle.
PATTERN = list("DDDDDDDDDDDDAAAA")

_NC_CACHE = {}
LAST_RESULTS = None
SPLIT_WAITS = True
TRACE = False
TRACE_KW = {}


def _split_multi_waits(nc):
    for f in nc.m.functions:
        for blk in f.blocks:
            insts = list(blk.instructions)
            out = []
            changed = False
            for inst in insts:
                si = inst.sync_info
                if si is not None and si.on_wait is not None and len(si.on_wait) > 1:
                    waits = list(si.on_wait)
                    for w in waits[:-1]:
                        d = mybir.InstDrain(
                            name=nc.get_next_instruction_name(),
                            ins=[], outs=[], bass_is_fusable=False)
                        d.engine = inst.engine
                        d.sync_info = mybir.SyncInfo(on_wait=[w], on_update=[])
                        out.append(d)
                    inst.sync_info = mybir.SyncInfo(
                        on_wait=[waits[-1]], on_update=list(si.on_update or []))
                    changed = True
                out.append(inst)
            if changed:
                blk.instructions = out


def build_nc():
    key = (tuple(PATTERN), tuple(CHUNK_SIZES), SPLIT_WAITS)
    if key in _NC_CACHE:
        return _NC_CACHE[key]

    assert len(PATTERN) == TILES_PER_DMA
    assert sum(CHUNK_SIZES) == N_TILES
    # A-tiles: contiguous even-length suffix of the 16-cycle; only
    # full-16 chunks carry A tiles.
    a_idx = [k for k, c in enumerate(PATTERN) if c == "A"]
    if a_idx:
        assert len(a_idx) % 2 == 0
        assert a_idx == list(range(a_idx[0], a_idx[0] + len(a_idx)))
        assert a_idx[0] % 2 == 0
    full16 = [i for i, sz in enumerate(CHUNK_SIZES) if sz == 16]
    # full-16 chunks must be contiguous (for the g_a strided view)
    assert full16 == list(range(full16[0], full16[0] + len(full16)))
    f16_start = sum(CHUNK_SIZES[:full16[0]])
    assign = []
    for sz in CHUNK_SIZES:
        assign += list(PATTERN) if sz == 16 else ["D"] * sz

    nc = bass.Bass()
    x = nc.declare_dram_parameter("x", [B_LOCAL, C], F32, isOutput=False)
    gin = nc.declare_dram_parameter("g", [P, N_TILES], F32, isOutput=False)
    out = nc.declare_dram_parameter("out", [1, 1], F32, isOutput=True)

    xv = x.rearrange("(p t) c -> p (t c)", p=P, t=N_TILES)
    AL = mybir.AluOpType
    AF = mybir.ActivationFunctionType
    n_a = sum(1 for c in assign if c == "A")
    n_pairs = N_TILES // 2                    # ones-stream pairs
    n_spairs = n_a // 2                       # sign-stream pairs

    with tile.TileContext(nc) as tc, ExitStack() as ctx:
        singles = ctx.enter_context(tc.tile_pool(name="singles", bufs=1))
        xpool = ctx.enter_context(tc.tile_pool(name="x", bufs=3))
        ppool = ctx.enter_context(tc.tile_pool(name="p", bufs=6))
        spool = ctx.enter_context(tc.tile_pool(name="s", bufs=4))
        psum = ctx.enter_context(tc.tile_pool(name="ps", bufs=1, space="PSUM"))

        G = singles.tile([P, N_TILES], F32)
        nc.sync.dma_start(G[:], gin[:])
        g16 = singles.tile([P, N_TILES], BF16)    # sign-pair weights
        nc.vector.tensor_copy(g16[:], G[:])
        NG = singles.tile([P, N_TILES], F32)      # -g (ACT bias)
        nc.vector.tensor_scalar_mul(NG[:], G[:], -1.0)
        ones_bf = singles.tile([P, 1], BF16)
        nc.vector.memset(ones_bf[:], 1.0)

        invdone = {}

        def emit_inv_block():
            GA = singles.tile([P, N_TILES], F32)
            nc.vector.tensor_scalar_add(GA[:], G[:], 0.1)
            INV = singles.tile([P, N_TILES], F32)
            nc.vector.reciprocal(INV[:], GA[:])
            if n_a:
                L = len(PATTERN)
                nf = len(full16)
                GV = G[:, f16_start:f16_start + nf * L].rearrange(
                    "p (u k) -> p u k", k=L)
                g_a = GV[:, :, a_idx[0]:a_idx[0] + len(a_idx)]
                rows_ga = singles.tile([P, 1], F32)
                nc.vector.tensor_reduce(rows_ga[:], g_a,
                                        axis=mybir.AxisListType.XY, op=AL.add)
                corr = singles.tile([P, 1], F32)
                nc.vector.tensor_scalar_mul(corr[:], rows_ga[:], 127.5)
            rows = singles.tile([P, 1], F32)
            nc.vector.tensor_reduce(rows[:], INV[:],
                                    axis=mybir.AxisListType.X, op=AL.add)
            if n_a:
                rows2 = singles.tile([P, 1], F32)
                nc.vector.tensor_tensor(out=rows2[:], in0=rows[:],
                                        in1=corr[:], op=AL.add)
                rows = rows2
            ones = singles.tile([P, 1], F32)
            nc.vector.memset(ones[:], 1.0)
            ps_fin = psum.tile([P, 8], F32, tag="fin")
            nc.tensor.matmul(ps_fin[:1, :1], ones[:], rows[:])
            fin = singles.tile([1, 1], F32)
            nc.vector.tensor_copy(fin[:], ps_fin[:1, :1])
            if n_a:
                w_a = singles.tile([2, 1], F32)
                nc.vector.memset(w_a[:], 0.0)
                nc.vector.memset(w_a[0:1, :], 0.5)
                w_b = singles.tile([2, 1], F32)
                nc.vector.memset(w_b[:], 0.5)
                nc.vector.memset(w_b[0:1, :], 0.0)
                invdone.update(fin=fin, w_a=w_a, w_b=w_b)
            else:
                invdone.update(fin=fin)

        ps_m = psum.tile([P, 512], F32, tag="m")     # ones-stream [1,512]
        if n_a:
            ps_s = psum.tile([P, 512], F32, tag="s")  # sign-stream [2,512]
        mm = 0
        sm = 0

        tile_base = 0
        for chunk, SZ in enumerate(CHUNK_SIZES):
            W = SZ * C
            xw = xpool.tile([P, 16 * C], F32, name="xw")
            # chunks 0+1 share the sync ring (FIFO -> chunk0 lands at
            # full bandwidth, starting compute sooner); alternate after
            eng = nc.sync if (chunk <= 1 or chunk % 2 == 0) else nc.scalar
            base = tile_base * C
            eng.dma_start(xw[:, :W], xv[:, base:base + W])

            pair = None
            spair = None
            u = 0
            su = 0
            for kk in range(SZ):
                t = tile_base + kk
                xt = xw[:, kk * C:(kk + 1) * C]
                gc = G[:, t:t + 1]
                if u == 0:
                    pair = ppool.tile([P, 2 * C], BF16, tag="pair",
                                      name="pair")
                if assign[t] == "D":
                    nc.vector.scalar_tensor_tensor(
                        out=pair[:, u * C:(u + 1) * C], in0=xt, scalar=gc,
                        in1=xt,
                        op0=mybir.AluOpType.is_gt, op1=mybir.AluOpType.mult)
                else:  # "A"
                    nc.scalar.activation(pair[:, u * C:(u + 1) * C], xt,
                                         AF.Relu, bias=NG[:, t:t + 1],
                                         scale=1.0)
                    if su == 0:
                        spair = spool.tile([P, 2 * C], BF16, tag="sp",
                                           name="sp")
                    nc.scalar.activation(spair[:, su * C:(su + 1) * C], xt,
                                         AF.Sign, bias=NG[:, t:t + 1],
                                         scale=1.0)
                    if su == 1:
                        nc.tensor.matmul(ps_s[:2, :], g16[:, t - 1:t + 1],
                                         spair[:],
                                         start=(sm == 0),
                                         stop=(sm == n_spairs - 1))
                        sm += 1
                    su ^= 1
                if u == 1:
                    nc.tensor.matmul(ps_m[:1, :], ones_bf[:], pair[:],
                                     start=(mm == 0),
                                     stop=(mm == n_pairs - 1))
                    mm += 1
                u ^= 1
            tile_base += SZ

        # ---- tail ------------------------------------------------------
        # ones-stream + sign-stream goods + fin -> one concat + reduce.
        emit_inv_block()
        fin = invdone["fin"]
        if n_a:
            w_a, w_b = invdone["w_a"], invdone["w_b"]
        TOT = singles.tile([1, 1025], F32)
        nc.vector.tensor_copy(TOT[:, 0:512], ps_m[:1, :])
        if n_a:
            # sign goods: row0[0:256], row1[256:512]; 0.5 factor is baked
            # into the selector weights.
            crow = singles.tile([2, 512], F32)
            nc.vector.tensor_copy(crow[:], ps_s[:2, :])
            ps_c2 = psum.tile([P, 1024], F32, tag="fin2")
            nc.tensor.matmul(ps_c2[:1, :512], w_a[:], crow[:])
            nc.tensor.matmul(ps_c2[:1, 512:], w_b[:], crow[:])
            nc.vector.tensor_copy(TOT[:, 512:768], ps_c2[:1, 0:C])
            nc.vector.tensor_copy(TOT[:, 768:1024], ps_c2[:1, 512 + C:1024])
        else:
            nc.vector.memset(TOT[:, 512:1024], 0.0)
        nc.vector.tensor_copy(TOT[:, 1024:1025], fin[:])
        acc = singles.tile([1, 1], F32)
        nc.vector.tensor_reduce(acc[:], TOT[:], axis=mybir.AxisListType.X,
                                op=AL.add)
        nc.sync.dma_start(out[:], acc[:])

    if SPLIT_WAITS:
        _split_multi_waits(nc)
    _NC_CACHE[key] = nc
    return nc


def _prep_inputs(rna_cell_out, rna_cell_label):
    x = np.asarray(rna_cell_out, dtype=np.float32)
    l = np.asarray(rna_cell_label).astype(np.int64)
    assert x.shape == (B, C) and l.shape == (B,)
    # Swap the true-class score into column 0 of every row (loss-
    # invariant layout prep; see module docstring).
    rows = np.arange(B)
    x2 = x.copy()
    vals = x[rows, l]
    x2[rows, l] = x[:, 0]
    x2[:, 0] = vals
    in_maps = []
    for i in range(N_CORES):
        xs = np.ascontiguousarray(x2[i * B_LOCAL:(i + 1) * B_LOCAL])
        gs = np.ascontiguousarray(xs[:, 0].reshape(P, N_TILES))
        in_maps.append({"x": xs, "g": gs})
    return in_maps


def kernel(rna_cell_out, rna_cell_label):
    global LAST_RESULTS
    nc = build_nc()
    in_maps = _prep_inputs(rna_cell_out, rna_cell_label)
    res = run_bass_kernel_spmd(nc, in_maps, list(range(N_CORES)),
                               trace=TRACE, **TRACE_KW)
    LAST_RESULTS = res
    parts = [float(res.results[i]["out"][0, 0]) for i in range(N_CORES)]
    loss = np.float32(np.sum(np.array(parts, dtype=np.float64)) / B)
    return np.array([loss], dtype=np.float32)


# revision 25
# speedup vs baseline: 1.0889x; 1.0889x over previous
"""Trainium2 Bass kernel for the CellLoss problem.

loss = mean_i [ 1/(x[i, l_i] + 0.1) + sum_j x[i,j] * (x[i,j] > x[i, l_i]) ]
with x: [131072, 256] f32, l: [131072] int labels in [0, 256).

Key reformulation: each row's loss is invariant under permuting that
row's 256 class scores, so the host swaps x[i, l_i] <-> x[i, 0] per row
(pure layout prep). The true-class score g then sits at column 0 of
every row: the per-tile gather pass disappears and g ships as a tiny
exact-f32 side tensor (1/(g+0.1) is ill-conditioned near g=-0.1; bf16 g
would cost ~1.4e-2 rel err, measured).

Pure data parallel across 8 NeuronCores (16384 rows each). Per core,
partition p owns rows [p*128, (p+1)*128); tile t is the [128, 256]
block of row p*128+t per partition. Everything reads the f32 x stream
directly - no bf16 cast pass exists (scalar_tensor_tensor runs at the
same 1x rate for f32 and bf16; only its OUTPUT is written bf16).

Margin per tile, engine chosen by PATTERN (cycle of 16):
 "D": DVE stt (x is_gt g) mult x -> masked-x tile (bf16 out).
 "A": ScalarE Relu(x-g) and Sign(x-g) (f32 in, bf16 out, exact f32
      bias); margin_A = sum relu + g*c with c from the sign sums:
      sum_j sign = 2c - 255 (the label ties exactly at 0).
TensorE accumulates the global sums in PSUM: ones^T @ [t|t+1] pairs of
masked-x/relu tiles -> ps_m [1,512]; [g_t|g_t+1]^T @ sign pairs ->
ps_s [2,512] whose junk half-rows are discarded at the tail via
selector-weight matmuls. inv = 1/(g+0.1) is computed up front (overlaps
the DMA fill). Host sums the 8 core partials / B.

DMA: 8 chunks of 2 MiB alternating the two HWDGE rings (sync/scalar).
"""

import numpy as np
from contextlib import ExitStack

import concourse.bass as bass
import concourse.mybir as mybir
import concourse.tile as tile
from concourse.bass_utils import run_bass_kernel_spmd

F32 = mybir.dt.float32
BF16 = mybir.dt.bfloat16

B, C = 131072, 256
N_CORES = 8
B_LOCAL = B // N_CORES          # 16384
P = 128
N_TILES = B_LOCAL // P          # 128
TILES_PER_DMA = 16              # pattern cycle length (A-capable chunks)
# chunk sizes in tiles: small first chunk (starts compute sooner), big
# middles, tiny tail chunks (cheap pipeline drain)
CHUNK_SIZES = [8] + [16] * 7 + [4, 4]

# margin engine per tile, cyclic ("D" DVE stt / "A" ScalarE relu+sign);
# A-tiles must come in adjacent pairs within the cycle.
PATTERN = list("DDDDDDDDDDDDAAAA")

_NC_CACHE = {}
LAST_RESULTS = None
SPLIT_WAITS = True
TRACE = False
TRACE_KW = {}


def _split_multi_waits(nc):
    for f in nc.m.functions:
        for blk in f.blocks:
            insts = list(blk.instructions)
            out = []
            changed = False
            for inst in insts:
                si = inst.sync_info
                if si is not None and si.on_wait is not None and len(si.on_wait) > 1:
                    waits = list(si.on_wait)
                    for w in waits[:-1]:
                        d = mybir.InstDrain(
                            name=nc.get_next_instruction_name(),
                            ins=[], outs=[], bass_is_fusable=False)
                        d.engine = inst.engine
                        d.sync_info = mybir.SyncInfo(on_wait=[w], on_update=[])
                        out.append(d)
                    inst.sync_info = mybir.SyncInfo(
                        on_wait=[waits[-1]], on_update=list(si.on_update or []))
                    changed = True
                out.append(inst)
            if changed:
                blk.instructions = out


def build_nc():
    key = (tuple(PATTERN), tuple(CHUNK_SIZES), SPLIT_WAITS)
    if key in _NC_CACHE:
        return _NC_CACHE[key]

    assert len(PATTERN) == TILES_PER_DMA
    assert sum(CHUNK_SIZES) == N_TILES
    # A-tiles: contiguous even-length suffix of the 16-cycle; only
    # full-16 chunks carry A tiles.
    a_idx = [k for k, c in enumerate(PATTERN) if c == "A"]
    if a_idx:
        assert len(a_idx) % 2 == 0
        assert a_idx == list(range(a_idx[0], a_idx[0] + len(a_idx)))
        assert a_idx[0] % 2 == 0
    full16 = [i for i, sz in enumerate(CHUNK_SIZES) if sz == 16]
    # full-16 chunks must be contiguous (for the g_a strided view)
    assert full16 == list(range(full16[0], full16[0] + len(full16)))
    f16_start = sum(CHUNK_SIZES[:full16[0]])
    assign = []
    for sz in CHUNK_SIZES:
        assign += list(PATTERN) if sz == 16 else ["D"] * sz

    nc = bass.Bass()
    x = nc.declare_dram_parameter("x", [B_LOCAL, C], F32, isOutput=False)
    gin = nc.declare_dram_parameter("g", [P, N_TILES], F32, isOutput=False)
    out = nc.declare_dram_parameter("out", [1, 1], F32, isOutput=True)

    xv = x.rearrange("(p t) c -> p (t c)", p=P, t=N_TILES)
    AL = mybir.AluOpType
    AF = mybir.ActivationFunctionType
    n_a = sum(1 for c in assign if c == "A")
    n_pairs = N_TILES // 2                    # ones-stream pairs
    n_spairs = n_a // 2                       # sign-stream pairs

    with tile.TileContext(nc) as tc, ExitStack() as ctx:
        singles = ctx.enter_context(tc.tile_pool(name="singles", bufs=1))
        xpool = ctx.enter_context(tc.tile_pool(name="x", bufs=3))
        ppool = ctx.enter_context(tc.tile_pool(name="p", bufs=6))
        spool = ctx.enter_context(tc.tile_pool(name="s", bufs=4))
        psum = ctx.enter_context(tc.tile_pool(name="ps", bufs=1, space="PSUM"))

        G = singles.tile([P, N_TILES], F32)
        nc.sync.dma_start(G[:], gin[:])
        g16 = singles.tile([P, N_TILES], BF16)    # sign-pair weights
        nc.vector.tensor_copy(g16[:], G[:])
        NG = singles.tile([P, N_TILES], F32)      # -g (ACT bias)
        nc.vector.tensor_scalar_mul(NG[:], G[:], -1.0)
        ones_bf = singles.tile([P, 1], BF16)
        nc.vector.memset(ones_bf[:], 1.0)

        invdone = {}

        def emit_inv_block():
            GA = singles.tile([P, N_TILES], F32)
            nc.vector.tensor_scalar_add(GA[:], G[:], 0.1)
            INV = singles.tile([P, N_TILES], F32)
            nc.vector.reciprocal(INV[:], GA[:])
            if n_a:
                L = len(PATTERN)
                nf = len(full16)
                GV = G[:, f16_start:f16_start + nf * L].rearrange(
                    "p (u k) -> p u k", k=L)
                g_a = GV[:, :, a_idx[0]:a_idx[0] + len(a_idx)]
                rows_ga = singles.tile([P, 1], F32)
                nc.vector.tensor_reduce(rows_ga[:], g_a,
                                        axis=mybir.AxisListType.XY, op=AL.add)
                corr = singles.tile([P, 1], F32)
                nc.vector.tensor_scalar_mul(corr[:], rows_ga[:], 127.5)
            rows = singles.tile([P, 1], F32)
            nc.vector.tensor_reduce(rows[:], INV[:],
                                    axis=mybir.AxisListType.X, op=AL.add)
            if n_a:
                rows2 = singles.tile([P, 1], F32)
                nc.vector.tensor_tensor(out=rows2[:], in0=rows[:],
                                        in1=corr[:], op=AL.add)
                rows = rows2
            ones = singles.tile([P, 1], F32)
            nc.vector.memset(ones[:], 1.0)
            ps_fin = psum.tile([P, 8], F32, tag="fin")
            nc.tensor.matmul(ps_fin[:1, :1], ones[:], rows[:])
            fin = singles.tile([1, 1], F32)
            nc.vector.tensor_copy(fin[:], ps_fin[:1, :1])
            if n_a:
                w_a = singles.tile([2, 1], F32)
                nc.vector.memset(w_a[:], 0.0)
                nc.vector.memset(w_a[0:1, :], 0.5)
                w_b = singles.tile([2, 1], F32)
                nc.vector.memset(w_b[:], 0.5)
                nc.vector.memset(w_b[0:1, :], 0.0)
                invdone.update(fin=fin, w_a=w_a, w_b=w_b)
            else:
                invdone.update(fin=fin)

        emit_inv_block()
        ps_m = psum.tile([P, 512], F32, tag="m")     # ones-stream [1,512]
        if n_a:
            ps_s = psum.tile([P, 512], F32, tag="s")  # sign-stream [2,512]
        mm = 0
        sm = 0

        tile_base = 0
        for chunk, SZ in enumerate(CHUNK_SIZES):
            W = SZ * C
            xw = xpool.tile([P, 16 * C], F32, name="xw")
            # chunks 0+1 share the sync ring (FIFO -> chunk0 lands at
            # full bandwidth, starting compute sooner); alternate after
            eng = nc.sync if (chunk <= 1 or chunk % 2 == 0) else nc.scalar
            base = tile_base * C
            eng.dma_start(xw[:, :W], xv[:, base:base + W])

            pair = None
            spair = None
            u = 0
            su = 0
            for kk in range(SZ):
                t = tile_base + kk
                xt = xw[:, kk * C:(kk + 1) * C]
                gc = G[:, t:t + 1]
                if u == 0:
                    pair = ppool.tile([P, 2 * C], BF16, tag="pair",
                                      name="pair")
                if assign[t] == "D":
                    nc.vector.scalar_tensor_tensor(
                        out=pair[:, u * C:(u + 1) * C], in0=xt, scalar=gc,
                        in1=xt,
                        op0=mybir.AluOpType.is_gt, op1=mybir.AluOpType.mult)
                else:  # "A"
                    nc.scalar.activation(pair[:, u * C:(u + 1) * C], xt,
                                         AF.Relu, bias=NG[:, t:t + 1],
                                         scale=1.0)
                    if su == 0:
                        spair = spool.tile([P, 2 * C], BF16, tag="sp",
                                           name="sp")
                    nc.scalar.activation(spair[:, su * C:(su + 1) * C], xt,
                                         AF.Sign, bias=NG[:, t:t + 1],
                                         scale=1.0)
                    if su == 1:
                        nc.tensor.matmul(ps_s[:2, :], g16[:, t - 1:t + 1],
                                         spair[:],
                                         start=(sm == 0),
                                         stop=(sm == n_spairs - 1))
                        sm += 1
                    su ^= 1
                if u == 1:
                    nc.tensor.matmul(ps_m[:1, :], ones_bf[:], pair[:],
                                     start=(mm == 0),
                                     stop=(mm == n_pairs - 1))
                    mm += 1
                u ^= 1
            tile_base += SZ

        # ---- tail ------------------------------------------------------
        # ones-stream + sign-stream goods + fin -> one concat + reduce.
        fin = invdone["fin"]
        if n_a:
            w_a, w_b = invdone["w_a"], invdone["w_b"]
        TOT = singles.tile([1, 1025], F32)
        nc.vector.tensor_copy(TOT[:, 0:512], ps_m[:1, :])
        if n_a:
            # sign goods: row0[0:256], row1[256:512]; 0.5 factor is baked
            # into the selector weights.
            crow = singles.tile([2, 512], F32)
            nc.vector.tensor_copy(crow[:], ps_s[:2, :])
            ps_c2 = psum.tile([P, 1024], F32, tag="fin2")
            nc.tensor.matmul(ps_c2[:1, :512], w_a[:], crow[:])
            nc.tensor.matmul(ps_c2[:1, 512:], w_b[:], crow[:])
            nc.vector.tensor_copy(TOT[:, 512:768], ps_c2[:1, 0:C])
            nc.vector.tensor_copy(TOT[:, 768:1024], ps_c2[:1, 512 + C:1024])
        else:
            nc.vector.memset(TOT[:, 512:1024], 0.0)
        nc.vector.tensor_copy(TOT[:, 1024:1025], fin[:])
        acc = singles.tile([1, 1], F32)
        nc.vector.tensor_reduce(acc[:], TOT[:], axis=mybir.AxisListType.X,
                                op=AL.add)
        nc.sync.dma_start(out[:], acc[:])

    if SPLIT_WAITS:
        _split_multi_waits(nc)
    _NC_CACHE[key] = nc
    return nc


def _prep_inputs(rna_cell_out, rna_cell_label):
    x = np.asarray(rna_cell_out, dtype=np.float32)
    l = np.asarray(rna_cell_label).astype(np.int64)
    assert x.shape == (B, C) and l.shape == (B,)
    # Swap the true-class score into column 0 of every row (loss-
    # invariant layout prep; see module docstring).
    rows = np.arange(B)
    x2 = x.copy()
    vals = x[rows, l]
    x2[rows, l] = x[:, 0]
    x2[:, 0] = vals
    in_maps = []
    for i in range(N_CORES):
        xs = np.ascontiguousarray(x2[i * B_LOCAL:(i + 1) * B_LOCAL])
        gs = np.ascontiguousarray(xs[:, 0].reshape(P, N_TILES))
        in_maps.append({"x": xs, "g": gs})
    return in_maps


def kernel(rna_cell_out, rna_cell_label):
    global LAST_RESULTS
    nc = build_nc()
    in_maps = _prep_inputs(rna_cell_out, rna_cell_label)
    res = run_bass_kernel_spmd(nc, in_maps, list(range(N_CORES)),
                               trace=TRACE, **TRACE_KW)
    LAST_RESULTS = res
    parts = [float(res.results[i]["out"][0, 0]) for i in range(N_CORES)]
    loss = np.float32(np.sum(np.array(parts, dtype=np.float64)) / B)
    return np.array([loss], dtype=np.float32)
